# revision 1
# baseline (speedup 1.0000x reference)
"""Trainium2 Bass kernel for nn_MultiHeadAttention_45062796870406.

Reference computation (per batch b, B=8 sharded 1-per-core across 8 cores):
    q = (query @ Wq).reshape(T, H, K);  k, v likewise
    logits[h,t',t] = q[t',h,:].k[t,h,:]/sqrt(K) + logit_offset[t',t,:] @ Wo_off[:,h] + bo_off[h]
    (mask is all-ones -> no-op; bo_off adds a per-(h,t') constant -> cancels in softmax)
    attn = softmax(logits, axis=t) @ v   -> out = attn.reshape(T, H*V) @ Wout + bout

Per-core design (T=1024, D=512, H=8, K=V=64, DM=512):
  - All matmuls bf16 with fp32 PSUM accumulation. 1/sqrt(K) folded into Wq on host.
  - x^T layouts produced by SWDGE cast-DMA (fp32->bf16) + HWDGE xbar DMA-transpose.
  - S' = S + off computed fused in PSUM: per 128-row t'-block, the score row
    [t', 8192] is laid out interleaved as pos = c*128 + h*16 + (t%16), c = t//16.
    S-matmuls (lhsT=qT_h [64,128], rhs=kT_h [64,64]) write strided 16-elem runs;
    the logit_offset matmuls (lhsT = transposed lo chunk [(16t,8o),128], rhs = W16
    host-built block-diag [128, (h,16t)]) accumulate on top.  One PSUM bank holds
    64 t x 8 h; an "octant" = 128 t = 2 banks.
  - exp on ScalarE (no max subtraction; logits are O(10) so exp is safe in fp32),
    P written bf16; PE-transpose P per (head, octant) -> PT chunks; PV matmuls
    lhsT = v chunk [128t, 64], rhs = PT [128t, 128t'] accumulate attnT [64, t'].
  - softmax denominators via ones-vector matmul over PT (row 0 of a psum bank),
    reciprocal on DVE, replicated across partitions with gpsimd.partition_broadcast,
    applied during attnT evacuation (fused divide).
  - final projection: lhsT = attnT chunks [64, 128], rhs = Wout chunks [64, 512],
    + bout (partition-broadcast once) during evacuation.
"""
import os
import sys

sys.path.insert(0, "/opt/trn_rl_repo")

import numpy as np
import ml_dtypes

import concourse.bass as bass
import concourse.mybir as mybir
import concourse.tile as tile
from concourse import bacc
from concourse.bass_utils import run_bass_kernel_spmd
import concourse.bass_utils as _bass_utils

if os.environ.get("K_LDW_OPT", "0") == "1" and not getattr(_bass_utils, "_ldw_patched", False):
    _orig_run_command = _bass_utils.run_command

    def _patched_run_command(argv, **kw):
        argv = ["--enable-ldw-opt=true" if a == "--enable-ldw-opt=false" else a
                for a in argv]
        return _orig_run_command(argv, **kw)

    _bass_utils.run_command = _patched_run_command
    _bass_utils._ldw_patched = True
from concourse.masks import make_identity

B, T, D = 8, 1024, 512
H, KD = 8, 64  # heads, head dim (K == V == 64)
DO, DM = 8, 512
TB = T // 128      # 8 t'-blocks
NOCT = T // 128    # 8 octants (t-chunks of 128) per t'-block
BF = mybir.dt.bfloat16
F32 = mybir.dt.float32

_cache = {}

TAIL_OCT = int(os.environ.get("K_TAIL_OCT", "5"))
PTS_BUFS = int(os.environ.get("K_PTS_BUFS", "3"))
SQ_BUFS = int(os.environ.get("K_SQ_BUFS", "3"))
P_BUFS = int(os.environ.get("K_P_BUFS", "2"))


def _build_program(debug=False, repeat=1):
    nc = bacc.Bacc()

    q_d = nc.dram_tensor("query", [T, D], F32, kind="ExternalInput")
    k_d = nc.dram_tensor("key", [T, D], F32, kind="ExternalInput")
    v_d = nc.dram_tensor("value", [T, D], F32, kind="ExternalInput")
    lo_d = nc.dram_tensor("lo", [T, T, DO], F32, kind="ExternalInput")
    wq_d = nc.dram_tensor("wq_bf", [D, D], BF, kind="ExternalInput")
    wk_d = nc.dram_tensor("wk_bf", [D, D], BF, kind="ExternalInput")
    wv_d = nc.dram_tensor("wv_bf", [D, D], BF, kind="ExternalInput")
    wo_d = nc.dram_tensor("wout_bf", [D, DM], BF, kind="ExternalInput")
    w16_d = nc.dram_tensor("w16", [128, 128], BF, kind="ExternalInput")
    bout_d = nc.dram_tensor("bout", [1, DM], F32, kind="ExternalInput")
    out_d = nc.dram_tensor("out", [T, DM], F32, kind="ExternalOutput")
    if debug:
        dbg = {
            "qt": nc.dram_tensor("dbg_qt", [64, H, T], BF, kind="ExternalOutput"),
            "kt": nc.dram_tensor("dbg_kt", [64, H, T], BF, kind="ExternalOutput"),
            "v": nc.dram_tensor("dbg_v", [128, TB, H, KD], BF, kind="ExternalOutput"),
            "xtq": nc.dram_tensor("dbg_xtq", [128, 4, TB, 128], BF, kind="ExternalOutput"),
            "p": nc.dram_tensor("dbg_p", [128, 1024], BF, kind="ExternalOutput"),
            "lot": nc.dram_tensor("dbg_lot", [128, 8, 128], BF, kind="ExternalOutput"),
            "pts": nc.dram_tensor("dbg_pts", [128, 8, 128], BF, kind="ExternalOutput"),
            "recip": nc.dram_tensor("dbg_recip", [1, 1024], F32, kind="ExternalOutput"),
            "att": nc.dram_tensor("dbg_att", [64, H, 128], BF, kind="ExternalOutput"),
        }

    with tile.TileContext(nc) as tc:
        with (
            tc.tile_pool(name="consts", bufs=1) as consts,
            tc.tile_pool(name="xc", bufs=int(os.environ.get("K_XC_BUFS", "12"))) as xc_pool,
            tc.tile_pool(name="xt", bufs=1) as xt_pool,
            tc.tile_pool(name="qkv", bufs=1) as qkv_pool,
            tc.tile_pool(name="lo", bufs=int(os.environ.get("K_LO_BUFS", "2"))) as lo_pool,
            tc.tile_pool(name="lot", bufs=int(os.environ.get("K_LOT_BUFS", "2"))) as lot_pool,
            tc.tile_pool(name="pb", bufs=P_BUFS) as p_pool,
            tc.tile_pool(name="pts", bufs=PTS_BUFS) as pts_pool,
            tc.tile_pool(name="att", bufs=2) as att_pool,
            tc.tile_pool(name="fo", bufs=2) as fo_pool,
            tc.tile_pool(name="sq", bufs=SQ_BUFS, space="PSUM") as sq_pool,
            tc.tile_pool(name="ptp", bufs=1, space="PSUM") as ptp_pool,
            tc.tile_pool(name="pvp", bufs=2, space="PSUM") as pv_pool,
            tc.tile_pool(name="smp", bufs=2, space="PSUM") as sm_pool,
        ):
            # ---------------- prologue: x loads first, then consts ----------------
            ident_f32 = consts.tile([128, 128], F32)
            make_identity(nc, ident_f32[:])
            ident_bf = consts.tile([128, 128], BF)
            make_identity(nc, ident_bf[:])

            # fp32 HWDGE loads (per t-block); PE transposes follow (PE is idle
            # in the prologue; SWDGE stays free for logit_offset prefetch)
            xT = {}
            xfs = {}
            for name, src_d in (("q", q_d), ("k", k_d), ("v", v_d)):
                xT[name] = xt_pool.tile([128, 4, TB, 128], BF, tag=f"xt_{name}", name=f"xt_{name}")
                xfs[name] = []
                for tb in range(TB):
                    xf = xc_pool.tile([128, D], F32, tag="xc", name="xc")
                    nc.sync.dma_start(out=xf, in_=src_d.ap()[tb * 128:(tb + 1) * 128, :])
                    xfs[name].append(xf)

            wq_sb = consts.tile([128, 4, D], BF)
            wk_sb = consts.tile([128, 4, D], BF)
            wv_sb = consts.tile([128, 4, D], BF)
            nc.sync.dma_start(out=wq_sb, in_=wq_d.ap().rearrange("(c p) d -> p c d", p=128))
            nc.sync.dma_start(out=wk_sb, in_=wk_d.ap().rearrange("(c p) d -> p c d", p=128))
            nc.sync.dma_start(out=wv_sb, in_=wv_d.ap().rearrange("(c p) d -> p c d", p=128))
            wout_sb = consts.tile([64, 8, DM], BF)
            nc.sync.dma_start(out=wout_sb, in_=wo_d.ap().rearrange("(h p) d -> p h d", p=64))
            w16_sb = consts.tile([128, 128], BF)
            nc.sync.dma_start(out=w16_sb, in_=w16_d.ap())
            ones_bf = consts.tile([128, 1], BF)
            nc.vector.memset(ones_bf, 1.0)
            bout_sb = consts.tile([1, DM], F32)
            nc.sync.dma_start(out=bout_sb, in_=bout_d.ap())
            bout_bc = consts.tile([128, DM], F32)
            nc.gpsimd.partition_broadcast(bout_bc[:], bout_sb[:])

            for name in ("q", "k", "v"):
                for tb in range(TB):
                    tp = sq_pool.tile([128, 512], F32, tag="sq", name="xtp")
                    tp4 = tp.rearrange("p (c r) -> p c r", r=128)
                    for c in range(4):
                        nc.tensor.transpose(
                            tp4[:, c, :], xfs[name][tb][:, c * 128:(c + 1) * 128],
                            ident_f32[:])
                    nc.vector.tensor_copy(xT[name][:, :, tb, :], tp4)

            # ---------------- projections ----------------
            # qT/kT: per head [64, 1024] bf16  (partitions 0-63)
            qt_sb = qkv_pool.tile([64, H, T], BF, tag="qt")
            kt_sb = qkv_pool.tile([64, H, T], BF, tag="kt")
            for name, wsb, dst in (("q", wq_sb, qt_sb), ("k", wk_sb, kt_sb)):
                for h in range(H):
                    for half in range(2):
                        ps = sq_pool.tile([128, 512], F32, tag="sq")
                        for c in range(4):
                            nc.tensor.matmul(
                                ps[0:64, :],
                                wsb[:, c, h * 64:(h + 1) * 64],
                                xT[name][:, c, :, :].rearrange("p tb t -> p (tb t)")[
                                    :, half * 512:(half + 1) * 512],
                                start=(c == 0), stop=(c == 3),
                            )
                        nc.scalar.copy(dst[:, h, half * 512:(half + 1) * 512], ps[0:64, :])

            # v: per t-block [128, (h, d)] bf16
            v_sb = qkv_pool.tile([128, TB, H, KD], BF, tag="v")
            for tb in range(TB):
                ps = sq_pool.tile([128, 512], F32, tag="sq")
                for c in range(4):
                    nc.tensor.matmul(
                        ps, xT["v"][:, c, tb, :], wv_sb[:, c, :],
                        start=(c == 0), stop=(c == 3),
                    )
                nc.scalar.copy(v_sb[:, tb, :, :].rearrange("p h d -> p (h d)"), ps[:])

            if debug:
                nc.sync.dma_start(out=dbg["qt"].ap(), in_=qt_sb)
                nc.sync.dma_start(out=dbg["kt"].ap(), in_=kt_sb)
                nc.sync.dma_start(out=dbg["v"].ap(), in_=v_sb)
                nc.sync.dma_start(out=dbg["xtq"].ap(), in_=xT["q"])

            # ---------------- main loop over t'-blocks ----------------
            def emit_tail(tpb, pv_ps, sm_ps):
                # normalization factors
                recip_sb = att_pool.tile([1, 1024], F32, tag="recip", name="recip")
                for j in range(2):
                    nc.vector.reciprocal(recip_sb[:, j * 512:(j + 1) * 512], sm_ps[j][:])
                rec_bc = att_pool.tile([64, 1024], F32, tag="recbc", name="recbc")
                nc.gpsimd.partition_broadcast(rec_bc[:], recip_sb[:])
                if debug and tpb == 0:
                    nc.sync.dma_start(out=dbg["recip"].ap(), in_=recip_sb)

                # attnT evacuation with fused divide
                att_sb = att_pool.tile([64, H, 128], BF, tag="att", name="att")
                for h in range(H):
                    nc.vector.tensor_mul(
                        att_sb[:, h, :],
                        pv_ps[h // 4][:, (h % 4) * 128:(h % 4 + 1) * 128],
                        rec_bc[:, h * 128:(h + 1) * 128],
                    )
                if debug and tpb == 0:
                    nc.sync.dma_start(out=dbg["att"].ap(), in_=att_sb)
                # final projection
                fo_ps = sq_pool.tile([128, 512], F32, tag="sq", name="fo_ps")
                for h in range(H):
                    nc.tensor.matmul(
                        fo_ps, att_sb[:, h, :], wout_sb[:, h, :],
                        start=(h == 0), stop=(h == H - 1),
                    )
                fo_sb = fo_pool.tile([128, DM], F32, tag="fo", name="fo_sb")
                nc.vector.tensor_add(fo_sb, fo_ps, bout_bc[:])
                nc.sync.dma_start(out=out_d.ap()[tpb * 128:(tpb + 1) * 128, :], in_=fo_sb)

            prev_tail = None
            for tpb_r in range(TB * repeat):
                tpb = tpb_r % TB
                pv_ps = [pv_pool.tile([64, 512], F32, tag="pv", name=f"pv{j}") for j in range(2)]
                sm_ps = [sm_pool.tile([1, 512], F32, tag="sm", name=f"sm{j}") for j in range(2)]

                lo2 = None
                for oct_ in range(NOCT):
                    half_i, oct_l = divmod(oct_, 4)
                    if oct_l == 0:
                        # load half-t'block of logit_offset (cast fp32->bf16)
                        lo2 = lo_pool.tile([128, 4, 1024], BF, tag="lo", name="lo2")
                        nc.gpsimd.dma_start(
                            out=lo2,
                            in_=lo_d.ap()[tpb * 128:(tpb + 1) * 128,
                                          half_i * 512:(half_i + 1) * 512, :]
                                .rearrange("p (c t) o -> p c (t o)", c=4),
                        )
                        lot2 = lot_pool.tile([128, 32, 128], BF, tag="lot", name="lot2")
                        nc.sync.dma_start_transpose(
                            lot2, lo2.rearrange("p c f -> p (c f)"))
                    lot_oct = lot2[:, oct_l * 8:(oct_l + 1) * 8, :]

                    p_oct = p_pool.tile([128, 1024], BF, tag="p", name="p_oct")
                    sqs = [sq_pool.tile([128, 512], F32, tag="sq", name=f"sq{q}")
                           for q in range(2)]
                    # S matmuls h-outer so consecutive mms share the stationary
                    # qT_h chunk (walrus ldw-opt elides redundant LDWEIGHTS)
                    for h in range(H):
                        for q in range(2):
                            sq3 = sqs[q].rearrange("p (c r) -> p c r", r=128)
                            nc.tensor.matmul(
                                sq3[:, :, h * 16:(h + 1) * 16],
                                qt_sb[:, h, tpb * 128:(tpb + 1) * 128],
                                kt_sb[:, h, oct_ * 128 + q * 64: oct_ * 128 + q * 64 + 64],
                                start=(h == 0), stop=False, skip_group_check=True,
                            )
                    for q in range(2):
                        # off matmuls accumulate on top (4 chunks of 16 t)
                        for cl in range(4):
                            nc.tensor.matmul(
                                sqs[q][:, cl * 128:(cl + 1) * 128],
                                lot_oct[:, q * 4 + cl, :],
                                w16_sb[:],
                                start=False, stop=(cl == 3), skip_group_check=True,
                            )
                        # exp; output de-interleaved to planar per-head layout:
                        # P_oct[t', h*128 + c*16 + ts] <- exp(sq[t', cl*128 + h*16 + ts])
                        p_view = p_oct.rearrange(
                            "p (h c ts) -> p c h ts", h=8, c=8, ts=16)[
                            :, 4 * q:4 * q + 4, :, :]
                        nc.scalar.activation(
                            p_view, sqs[q][:],
                            mybir.ActivationFunctionType.Exp,
                        )

                    # transpose P per head -> PT psum bank -> SBUF
                    ptp = ptp_pool.tile([128, 8, 128], BF, tag="ptp", name="ptp")
                    for h in range(H):
                        nc.tensor.transpose(
                            ptp[:, h, :], p_oct[:, h * 128:(h + 1) * 128], ident_bf[:],
                        )
                    pts = pts_pool.tile([128, 8, 128], BF, tag="pts", name="pts")
                    nc.vector.tensor_copy(pts, ptp)
                    if debug and tpb == 0 and oct_ == 0:
                        nc.sync.dma_start(out=dbg["p"].ap(), in_=p_oct)
                        nc.sync.dma_start(out=dbg["lot"].ap(), in_=lot_oct)
                        nc.sync.dma_start(out=dbg["pts"].ap(), in_=pts)

                    # PV accumulation + sums
                    for h in range(H):
                        # start=True clears has_written for the WHOLE bank, so
                        # only the first head of each 4-head bank may set it.
                        nc.tensor.matmul(
                            pv_ps[h // 4][:, (h % 4) * 128:(h % 4 + 1) * 128],
                            v_sb[:, oct_, h, :],
                            pts[:, h, :],
                            start=(oct_ == 0 and h % 4 == 0),
                            stop=(oct_ == NOCT - 1),
                            skip_group_check=True,
                        )
                    for j in range(2):
                        nc.tensor.matmul(
                            sm_ps[j],
                            ones_bf[:],
                            pts[:, j * 4:(j + 1) * 4, :].rearrange("p h t -> p (h t)"),
                            start=(oct_ == 0), stop=(oct_ == NOCT - 1),
                            skip_group_check=True,
                        )

                    # software-pipeline: previous t'block's tail after 2 octants
                    if oct_ == TAIL_OCT and prev_tail is not None:
                        emit_tail(*prev_tail)
                        prev_tail = None

                prev_tail = (tpb, pv_ps, sm_ps)
            emit_tail(*prev_tail)

    nc.compile()
    return nc


def _prep_weights(Wq, Wk, Wv, Wo_off, Wout, bout):
    bf = ml_dtypes.bfloat16
    wq_bf = (np.asarray(Wq, np.float32) / np.sqrt(KD).astype(np.float32)).astype(bf)
    wk_bf = np.asarray(Wk, np.float32).astype(bf)
    wv_bf = np.asarray(Wv, np.float32).astype(bf)
    wout_bf = np.asarray(Wout, np.float32).astype(bf)
    w16 = np.zeros((128, 128), np.float32)
    wo = np.asarray(Wo_off, np.float32)  # [DO, H]
    for ts in range(16):
        for o in range(DO):
            for h in range(H):
                w16[ts * 8 + o, h * 16 + ts] = wo[o, h]
    w16 = w16.astype(bf)
    bout_f = np.asarray(bout, np.float32).reshape(1, DM)
    return wq_bf, wk_bf, wv_bf, wout_bf, w16, bout_f


def kernel(query, key, value, logit_offset, mask=None, Wq=None, Wk=None, Wv=None,
           Wo_off=None, bo_off=None, Wout=None, bout=None, **_unused):
    # mask is all-ones in this problem (fill: ones) -> no-op.
    # bo_off adds a constant per (h, t') row -> cancels in softmax.
    query = np.asarray(query, np.float32)
    key = np.asarray(key, np.float32)
    value = np.asarray(value, np.float32)
    logit_offset = np.asarray(logit_offset, np.float32)
    wq_bf, wk_bf, wv_bf, wout_bf, w16, bout_f = _prep_weights(
        Wq, Wk, Wv, Wo_off, Wout, bout)

    if "nc" not in _cache:
        _cache["nc"] = _build_program()
    nc = _cache["nc"]

    in_maps = []
    for b in range(B):
        in_maps.append({
            "query": query[b], "key": key[b], "value": value[b],
            "lo": logit_offset[b],
            "wq_bf": wq_bf, "wk_bf": wk_bf, "wv_bf": wv_bf,
            "wout_bf": wout_bf, "w16": w16, "bout": bout_f,
        })
    res = run_bass_kernel_spmd(nc, in_maps, core_ids=list(range(B)))
    out = np.stack([res.results[b]["out"] for b in range(B)], axis=0)
    return out.astype(np.float32)


def run_traced(query, key, value, logit_offset, mask=None, **weights):
    """Like kernel() but returns (out, BassKernelResults) with trace enabled."""
    query = np.asarray(query, np.float32)
    key = np.asarray(key, np.float32)
    value = np.asarray(value, np.float32)
    logit_offset = np.asarray(logit_offset, np.float32)
    wq_bf, wk_bf, wv_bf, wout_bf, w16, bout_f = _prep_weights(
        weights["Wq"], weights["Wk"], weights["Wv"], weights["Wo_off"],
        weights["Wout"], weights["bout"])
    if "nc" not in _cache:
        _cache["nc"] = _build_program()
    nc = _cache["nc"]
    in_maps = []
    for b in range(B):
        in_maps.append({
            "query": query[b], "key": key[b], "value": value[b],
            "lo": logit_offset[b],
            "wq_bf": wq_bf, "wk_bf": wk_bf, "wv_bf": wv_bf,
            "wout_bf": wout_bf, "w16": w16, "bout": bout_f,
        })
    res = run_bass_kernel_spmd(nc, in_maps, core_ids=list(range(B)), trace=True)
    out = np.stack([res.results[b]["out"] for b in range(B)], axis=0)
    return out.astype(np.float32), res

